# revision 8
# baseline (speedup 1.0000x reference)
"""Head-sharded collective-free causal self-attention for 8 TRN2 cores.

Sharding: core c -> batch b = c//2, head group g = c%2 (heads [8g, 8g+8)).
Each core computes q/k/v for its 8 heads over ALL 2048 tokens of its batch
(no k/v duplication), runs causal attention with an SPMD-uniform trapezoid
(query chunks of 256 attend to kv in [0, 256*(qc+1)), diagonal chunks
masked), and a Megatron row-parallel projection producing PARTIAL sums of
the full [2048, 1024] output.  The pair-reduce of the two partials per
batch happens on the host at unshard time (collective-free).

All matmuls stream bf16 at 1 column/cycle, so PE time = streamed columns:
this design cuts k/v production 2x and S/AV work 25% vs computing the full
causal square.  Softmax normalizer via a ones column in the augmented v
(row 64 of the AV PSUM = sum_k P); no max-subtraction (logits are O(5)).

Engine balance: PE ~240us is the bottleneck; exp (Activation) ~150us;
everything else (bias adds, masks, normalize) spread over DVE/Pool.  QKV
production for slabs 1-3 and the projection are issued through a "drip"
queue rationed into the attention chains so the PE stays fed while the
Activation engine works through the exps; normalization is pipelined one
chain behind its S/AV issue so its reciprocal never stalls the PE.
"""

import numpy as np
import ml_dtypes

import concourse.bass as bass
import concourse.mybir as mybir
import concourse.tile as tile
from concourse import bacc
from concourse import bass_utils

F32 = mybir.dt.float32
BF16 = mybir.dt.bfloat16

B, T, C = 4, 2048, 1024
NH, HS = 16, 64
NCORES = 8
P = 128
KO = C // P              # 8 contraction chunks over C
FW = 512                 # feature width per core (8 heads x 64)
NF = FW // P             # 4 feature chunks (head pairs)
SLAB = 512               # token slab for QKV production
NSLAB = T // SLAB        # 4
QCW = 256                # attention query-chunk width
NQC = T // QCW           # 8
KC = 128                 # kv chunk width (PSUM partition)
NTC = T // P             # 16 token chunks for v
PKO = FW // P            # 4 proj contraction chunks

# drips (dense PE work units) injected per attention chain, by query chunk
BUD = {0: 2, 1: 2, 2: 1, 3: 1, 4: 2, 5: 2, 6: 3, 7: 4}


def build_graph():
    nc = bacc.Bacc(
        "TRN2",
        target_bir_lowering=False,
        debug=False,
        enable_asserts=True,
        num_devices=NCORES,
    )

    xt = nc.dram_tensor("xt", [C, T], BF16, kind="ExternalInput").ap()
    w_q = nc.dram_tensor("w_q", [C, FW], BF16, kind="ExternalInput").ap()
    w_k = nc.dram_tensor("w_k", [C, FW], BF16, kind="ExternalInput").ap()
    w_v = nc.dram_tensor("w_v", [C, FW], BF16, kind="ExternalInput").ap()
    w_p = nc.dram_tensor("w_p", [FW, C], BF16, kind="ExternalInput").ap()
    b_q = nc.dram_tensor("b_q", [FW], F32, kind="ExternalInput").ap()
    b_k = nc.dram_tensor("b_k", [FW], F32, kind="ExternalInput").ap()
    b_v = nc.dram_tensor("b_v", [FW], F32, kind="ExternalInput").ap()
    b_p = nc.dram_tensor("b_p", [C], F32, kind="ExternalInput").ap()
    mask = nc.dram_tensor("mask", [P, KC], BF16, kind="ExternalInput").ap()
    out = nc.dram_tensor("out", [T, C], F32, kind="ExternalOutput").ap()

    xt_t = xt.rearrange("(ko p) t -> p ko t", p=P)       # [128, 8, 2048]
    wq_t = w_q.rearrange("(ko p) f -> p ko f", p=P)      # [128, 8, 512]
    wk_t = w_k.rearrange("(ko p) f -> p ko f", p=P)
    wv_t = w_v.rearrange("(ko p) f -> p ko f", p=P)
    wp_t = w_p.rearrange("(ko p) f -> p ko f", p=P)      # [128, 4, 1024]
    bq_t = b_q.rearrange("(f p) -> p f", p=P)            # [128, 4]
    bk_t = b_k.rearrange("(f p) -> p f", p=P)

    EXP = mybir.ActivationFunctionType.Exp
    ADD = mybir.AluOpType.add
    MUL = mybir.AluOpType.mult

    with tile.TileContext(nc) as tc:
        with (
            tc.tile_pool(name="const", bufs=1) as const,
            tc.tile_pool(name="w", bufs=1) as w_pool,
            tc.tile_pool(name="xs", bufs=1) as xs_pool,
            tc.tile_pool(name="big", bufs=1) as big,
            tc.tile_pool(name="pexp", bufs=18) as p_pool,
            tc.tile_pool(name="small", bufs=3) as small,
            tc.tile_pool(name="outsb", bufs=3) as out_pool,
            tc.tile_pool(name="mm_ps", bufs=3, space="PSUM") as mm_ps,
            tc.tile_pool(name="st_ps", bufs=3, space="PSUM") as st_ps,
            tc.tile_pool(name="y_ps", bufs=2, space="PSUM") as y_ps,
        ):
            # ---- small constants ----
            ones_row = const.tile([1, P], BF16)
            nc.vector.memset(ones_row[:], 1.0)
            bq_sb = const.tile([P, NF], F32)
            nc.sync.dma_start(bq_sb[:], bq_t)
            bk_sb = const.tile([P, NF], F32)
            nc.sync.dma_start(bk_sb[:], bk_t)
            m_sb = const.tile([P, 1, KC], BF16)
            nc.sync.dma_start(m_sb[:, 0, :], mask)

            # v/proj bias broadcast across partitions via ones-column matmul
            bv_row = const.tile([1, FW], F32)
            nc.sync.dma_start(bv_row[:], b_v[None, :])
            bv_row16 = const.tile([1, FW], BF16)
            nc.vector.tensor_copy(bv_row16[:], bv_row[:])
            bv_bc = const.tile([P, 8, HS], F32)
            bv_ps = mm_ps.tile([P, 8, HS], F32, tag="mm", name="bv_ps")
            nc.tensor.matmul(bv_ps[:, :, :], ones_row[:], bv_row16[:],
                             start=True, stop=True)
            nc.vector.tensor_copy(bv_bc[:], bv_ps[:])

            bp_row = const.tile([1, C], F32)
            nc.sync.dma_start(bp_row[:], b_p[None, :])
            bp_row16 = const.tile([1, C], BF16)
            nc.vector.tensor_copy(bp_row16[:], bp_row[:])
            bp_bc = const.tile([P, C], F32)
            for half in range(2):
                bp_ps = mm_ps.tile([P, SLAB], F32, tag="mm", name=f"bp_ps{half}")
                nc.tensor.matmul(bp_ps[:], ones_row[:],
                                 bp_row16[:, half * SLAB:(half + 1) * SLAB],
                                 start=True, stop=True)
                nc.vector.tensor_copy(bp_bc[:, half * SLAB:(half + 1) * SLAB],
                                      bp_ps[:])

            # ---- weights + x slabs (all DMAs issued up front, in the order
            # compute needs them) ----
            w_k_sb = w_pool.tile([P, KO, FW], BF16, name="w_k_sb")
            for fq in range(NF):
                nc.sync.dma_start(w_k_sb[:, :, fq * P:(fq + 1) * P],
                                  wk_t[:, :, fq * P:(fq + 1) * P])
            xs = []
            for s in range(NSLAB):
                slab = xs_pool.tile([P, KO, SLAB], BF16, name=f"xs{s}")
                xs.append(slab)
            for kd in range(KO):
                nc.sync.dma_start(xs[0][:, kd, :], xt_t[:, kd, 0:SLAB])
            w_v_sb = w_pool.tile([P, KO, FW], BF16, name="w_v_sb")
            for fq in range(NF):
                nc.sync.dma_start(w_v_sb[:, :, fq * P:(fq + 1) * P],
                                  wv_t[:, :, fq * P:(fq + 1) * P])
            w_q_sb = w_pool.tile([P, KO, FW], BF16, name="w_q_sb")
            for fq in range(NF):
                nc.sync.dma_start(w_q_sb[:, :, fq * P:(fq + 1) * P],
                                  wq_t[:, :, fq * P:(fq + 1) * P])
            for s in range(1, NSLAB):
                for kd in range(KO):
                    nc.sync.dma_start(xs[s][:, kd, :],
                                      xt_t[:, kd, s * SLAB:(s + 1) * SLAB])
            w_p_sb = w_pool.tile([P, PKO, C], BF16, name="w_p_sb")
            for fq in range(2 * NF):
                nc.sync.dma_start(w_p_sb[:, :, fq * P:(fq + 1) * P],
                                  wp_t[:, :, fq * P:(fq + 1) * P])

            # ---- persistent activations ----
            qT = big.tile([P, NF, T], BF16, name="qT")
            kT = big.tile([P, NF, T], BF16, name="kT")
            v_aug = big.tile([P, NTC, 8, HS + 1], BF16, name="v_aug")
            yT = big.tile([P, NF, T], BF16, name="yT")
            for h in range(8):
                nc.gpsimd.memset(v_aug[:, :, h, HS:HS + 1], 1.0)

            # ---- dense (exp-free) PE work: QKV production + projection ----
            def make_kq(kind, s, fc):
                w_sb, bias_sb, dstT = {
                    "k": (w_k_sb, bk_sb, kT),
                    "q": (w_q_sb, bq_sb, qT),
                }[kind]

                def go():
                    ps = mm_ps.tile([P, SLAB], F32, tag="mm",
                                    name=f"{kind}ps_{s}_{fc}")
                    for k0 in range(KO):
                        nc.tensor.matmul(
                            ps[:], w_sb[:, k0, fc * P:(fc + 1) * P],
                            xs[s][:, k0, :],
                            start=(k0 == 0), stop=(k0 == KO - 1),
                        )
                    nc.vector.tensor_scalar_add(
                        dstT[:, fc, s * SLAB:(s + 1) * SLAB], ps[:],
                        bias_sb[:, fc:fc + 1])
                return go

            def make_v(s, t4):
                def go():
                    tc_g = s * (SLAB // P) + t4
                    ps = mm_ps.tile([P, 8, HS], F32, tag="mm",
                                    name=f"vps_{tc_g}")
                    for k0 in range(KO):
                        nc.tensor.matmul(
                            ps[:, :, :], xs[s][:, k0, t4 * P:(t4 + 1) * P],
                            w_v_sb[:, k0, :],
                            start=(k0 == 0), stop=(k0 == KO - 1),
                        )
                    nc.vector.tensor_tensor(
                        v_aug[:, tc_g, :, 0:HS], ps[:], bv_bc[:], ADD)
                return go

            def make_proj(tm, nn):
                def go():
                    ps = mm_ps.tile([P, SLAB], F32, tag="mm",
                                    name=f"pps_{tm}_{nn}")
                    for k0 in range(PKO):
                        nc.tensor.matmul(
                            ps[:], yT[:, k0, tm * P:(tm + 1) * P],
                            w_p_sb[:, k0, nn * SLAB:(nn + 1) * SLAB],
                            start=(k0 == 0), stop=(k0 == PKO - 1),
                        )
                    osb = out_pool.tile([P, SLAB], F32, tag="osb")
                    nc.vector.tensor_tensor(
                        osb[:], ps[:], bp_bc[:, nn * SLAB:(nn + 1) * SLAB], ADD)
                    nc.sync.dma_start(
                        out[tm * P:(tm + 1) * P, nn * SLAB:(nn + 1) * SLAB],
                        osb[:])
                return go

            dense = []

            def drip(n):
                for _ in range(n):
                    if dense:
                        dense.pop(0)()

            def qkv_slab_ops(s):
                ops = [make_kq("k", s, fc) for fc in range(NF)]
                ops += [make_v(s, t4) for t4 in range(SLAB // P)]
                ops += [make_kq("q", s, fc) for fc in range(NF)]
                return ops

            # slab 0 issued directly; slabs 1-3 go into the drip queue
            for op in qkv_slab_ops(0):
                op()
            for s in range(1, NSLAB):
                dense.extend(qkv_slab_ops(s))

            # ---- attention: head pairs packed per feature chunk ----
            tril_bc = m_sb[:, 0:1, :].to_broadcast((P, 2, KC))

            def attn_sav(fc, qc):
                budget = BUD[qc]
                E = 2 * qc + 2
                # each open PSUM accumulation group needs its own bank:
                # start=True zeroes the full 2KB region
                yps_e = y_ps.tile([HS + 1, QCW], F32, tag="y",
                                  name=f"ye_{fc}_{qc}")
                yps_o = y_ps.tile([HS + 1, QCW], F32, tag="y",
                                  name=f"yo_{fc}_{qc}")
                pexps = []
                drip(1)
                for kc in range(E):
                    even_diag = kc == E - 2
                    odd_diag = kc == E - 1
                    q0 = KC if odd_diag else 0
                    pexp = p_pool.tile([P, 2, QCW], BF16, tag="p")
                    if odd_diag:
                        nc.gpsimd.memset(pexp[:, :, 0:KC], 0.0)
                    for j, hp in ((0, 0), (1, HS)):
                        stps = st_ps.tile([P, QCW], F32, tag="st")
                        nc.tensor.matmul(
                            stps[:, q0:QCW],
                            kT[hp:hp + HS, fc, kc * KC:(kc + 1) * KC],
                            qT[hp:hp + HS, fc, qc * QCW + q0:(qc + 1) * QCW],
                            start=True, stop=True,
                            tile_position=(hp, 0),
                        )
                        nc.scalar.activation(pexp[:, j, q0:QCW],
                                             stps[:, q0:QCW],
                                             EXP, scale=1.0 / np.sqrt(HS))
                    if even_diag:
                        nc.vector.tensor_tensor(
                            pexp[:, :, 0:KC], pexp[:, :, 0:KC], tril_bc, MUL)
                    elif odd_diag:
                        nc.vector.tensor_tensor(
                            pexp[:, :, KC:QCW], pexp[:, :, KC:QCW], tril_bc, MUL)
                    pexps.append(pexp)
                    if budget >= 2 and kc == E // 2:
                        drip(1)
                if budget >= 3:
                    drip(1)
                for kc in range(E):
                    for j, yps in ((0, yps_e), (1, yps_o)):
                        nc.tensor.matmul(
                            yps[:],
                            v_aug[:, kc, 2 * fc + j, :],
                            pexps[kc][:, j, :],
                            start=(kc == 0), stop=(kc == E - 1),
                        )
                    if budget >= 4 and kc == E // 2:
                        drip(1)
                # evict to SBUF right away so the PSUM banks free up
                y_sb = small.tile([HS + 1, 2, QCW], F32, tag="y_sb")
                nc.vector.tensor_copy(y_sb[:, 0, :], yps_e[:])
                nc.vector.tensor_copy(y_sb[:, 1, :], yps_o[:])

                def norm():
                    # row 64 of y_sb = sum_k exp -> reciprocal -> broadcast
                    recip = small.tile([1, 2, QCW], BF16, tag="recip")
                    with nc.allow_low_precision(
                            reason="bf16 softmax normalizer within tolerance"):
                        nc.vector.reciprocal(recip[:], y_sb[HS:HS + 1, :, :])
                    bc_sb = small.tile([P, 2, QCW], BF16, tag="bc_sb")
                    nc.gpsimd.partition_broadcast(bc_sb[:], recip[:])
                    for j, hp in ((0, 0), (1, HS)):
                        nc.vector.tensor_tensor(
                            yT[hp:hp + HS, fc, qc * QCW:(qc + 1) * QCW],
                            y_sb[0:HS, j, :], bc_sb[0:HS, j, :], MUL)
                return norm

            pending = None
            for qc in range(NQC):
                for fc in range(NF):
                    nxt = attn_sav(fc, qc)
                    if pending is not None:
                        pending()
                    if fc == 0 and qc > 0:
                        for tm in (2 * (qc - 1), 2 * (qc - 1) + 1):
                            for nn in range(2):
                                dense.append(make_proj(tm, nn))
                    pending = nxt
            pending()
            drip(len(dense))
            for tm in (T // P - 2, T // P - 1):
                for nn in range(2):
                    make_proj(tm, nn)()

    nc.compile()
    return nc


_NC_CACHE = None


def _get_nc():
    global _NC_CACHE
    if _NC_CACHE is None:
        _NC_CACHE = build_graph()
    return _NC_CACHE


def make_in_maps(x, W_attn, b_attn, W_proj, b_proj):
    x = np.asarray(x, dtype=np.float32)
    W_attn = np.asarray(W_attn, dtype=np.float32)
    b_attn = np.asarray(b_attn, dtype=np.float32)
    W_proj = np.asarray(W_proj, dtype=np.float32)
    b_proj = np.asarray(b_proj, dtype=np.float32)

    bf = ml_dtypes.bfloat16
    kv = np.arange(KC)
    q = np.arange(KC)
    tril = np.ascontiguousarray((q[None, :] >= kv[:, None]).astype(bf))

    xts = [np.ascontiguousarray(x[b].T).astype(bf) for b in range(B)]

    in_maps = []
    for c in range(NCORES):
        b, g = c // 2, c % 2
        sl = slice(FW * g, FW * g + FW)
        in_maps.append({
            "xt": xts[b],
            "w_q": np.ascontiguousarray(W_attn[:, sl]).astype(bf),
            "w_k": np.ascontiguousarray(W_attn[:, C:][:, sl]).astype(bf),
            "w_v": np.ascontiguousarray(W_attn[:, 2 * C:][:, sl]).astype(bf),
            "w_p": np.ascontiguousarray(W_proj[sl, :]).astype(bf),
            "b_q": np.ascontiguousarray(b_attn[sl]),
            "b_k": np.ascontiguousarray(b_attn[C:][sl]),
            "b_v": np.ascontiguousarray(b_attn[2 * C:][sl]),
            "b_p": (b_proj if g == 0 else np.zeros_like(b_proj)),
            "mask": tril,
        })
    return in_maps


def kernel(x, W_attn, b_attn, W_proj, b_proj):
    nc = _get_nc()
    in_maps = make_in_maps(x, W_attn, b_attn, W_proj, b_proj)
    res = bass_utils.run_bass_kernel_spmd(
        nc, in_maps, core_ids=list(range(NCORES)), trace=False,
    )
    out_full = np.empty((B, T, C), dtype=np.float32)
    for b in range(B):
        out_full[b] = res.results[2 * b]["out"] + res.results[2 * b + 1]["out"]
    kernel.last_results = res
    return out_full
